# revision 1
# baseline (speedup 1.0000x reference)
"""AttentionPooling (query position 0 only) — Trainium2 Bass/Tile kernel, v2.

Math (per batch n, heads h=8, dh=32, D=256, T=4096):
    q0 = v[n,0,:] @ W_q + b_q
    scores[t,h] = (1/16) * q0[head h slice] . k[t, head h slice],  k = v@W_k + b_k
Folded:  fq[din,h] = sum_{j in head h} W_k[din,j] * q0[j] / 16
         scores[t,h] = sum_din v[t,din] * fq[din,h]  (+ c[h], which is constant
         over t and cancels exactly in softmax -> dropped)
    out[h,:] = sum_t softmax_t(scores[:,h]) * v[t,:]   keep cols [32h:32h+32]

Layout choices (4x over the fp32 v1 kernel, 258us -> ~64us):
  * v is uploaded from the host already in bf16, padded with a ones
    column at index 256.  Halves HBM traffic (the DMA engines cap at
    ~21GB/s x16 ~ the per-core HBM limit) and deletes the on-device
    fp32->bf16 convert stage; the ones column feeds the softmax-
    denominator column of the value matmul (no accum readout/reduce).
  * All heavy matmuls in bf16: FWL doubles LDWEIGHTS rate and the moving
    operand streams at 1 col/cycle (fp32 is 4x slower + no FWL).
  * Scores are produced directly transposed, sT[t,h], via vT-stationary
    matmuls: exp then runs on all 128 partitions ([128,32] instead of
    [8,512]) and the e-transpose pass disappears.  The k-projection bias
    term is constant over t and cancels exactly in softmax -> dropped.
  * Chunk DMA pairs two 512-token chunks with p-major packing
    (t = t0 + 8*p + jj) so each partition receives one contiguous ~4KB
    HBM segment -> large DMA packets.  All consumers use the same
    permuted t-order; the reduction over t is permutation-invariant.
  * 5-deep pair prefetch hides the ~0.9us DMA-semaphore propagation;
    PE warmup matmuls at startup open the HAM clock-gate (1.2->2.4GHz)
    before the real stream arrives.
  * The value stage is emitted one chunk late: V(i) waits on exp(i), and
    in the PE's strict FIFO it would otherwise block chunk i+1's
    transposes while waiting (~0.5us/chunk once DMA stops pacing).

Sharding: data-parallel over N across 8 cores (4 batches per core), no
collectives.
"""

import sys

if "/opt/trn_rl_repo" not in sys.path:
    sys.path.insert(0, "/opt/trn_rl_repo")

import numpy as np

N_FULL, T, DIN = 32, 4096, 256
H = 8
NCORES = 8
NB = N_FULL // NCORES  # batches per core
TC = 512               # t-chunk processed per iteration
NJ = TC // 128         # 128-row blocks per chunk
NCH = T // TC          # chunks per batch
GCH = NB * NCH         # chunks per core
SCALE = 1.0 / 16.0     # 1/sqrt(D)

_CACHE = {}


def _build():
    from contextlib import ExitStack

    import concourse.mybir as mybir
    from concourse import bacc
    from concourse.masks import make_identity
    from concourse.tile import TileContext

    fp32 = mybir.dt.float32
    bf16 = mybir.dt.bfloat16
    AF = mybir.ActivationFunctionType

    nc = bacc.Bacc(None, target_bir_lowering=False)
    # v is fed pre-converted to bf16 on the host, padded with a ones column
    # at index 256 (feeds the softmax-denominator column of the value
    # matmul).  Halves HBM traffic — the kernel computes in bf16 anyway.
    v_ext = nc.declare_dram_parameter("v", [NB, T, DIN + 1], bf16, isOutput=False)
    w_ext = nc.declare_dram_parameter("W_qk", [DIN, 2 * DIN], bf16, isOutput=False)
    b_ext = nc.declare_dram_parameter("b_qk", [2 * DIN], fp32, isOutput=False)
    # full per-head pooled vectors; host extracts the per-head 32-col slices
    out_ext = nc.declare_dram_parameter("out", [NB, H, DIN], fp32, isOutput=True)

    with TileContext(nc) as tc:
        with ExitStack() as ctx:
            const = ctx.enter_context(tc.tile_pool(name="const", bufs=1))

            ident = const.tile([128, 128], fp32)
            make_identity(nc, ident)
            ident_bf = const.tile([128, 128], bf16)
            nc.vector.tensor_copy(out=ident_bf, in_=ident)

            # W_k first: phase0's longest chain (wkT transpose -> fq) starts
            # on W_k alone; W_q is not needed until the later q0 matmuls
            wk_sb = const.tile([128, 2, 256], bf16)
            nc.sync.dma_start(
                out=wk_sb, in_=w_ext[:, 256:512].rearrange("(kc p) d -> p kc d", p=128)
            )
            wq_sb = const.tile([128, 2, 256], bf16)
            nc.sync.dma_start(
                out=wq_sb, in_=w_ext[:, 0:256].rearrange("(kc p) d -> p kc d", p=128)
            )
            # b_q natural: [1, 256] — one contiguous descriptor (the strided
            # per-element gather costs ~256 4-byte DMA packets at startup)
            bqn_sb = const.tile([1, 256], fp32)
            nc.sync.dma_start(
                out=bqn_sb, in_=b_ext[0:256].rearrange("(o d) -> o d", o=1)
            )
            # v[:, 0, :] natural: [NB, 257] bf16 — NB contiguous descriptors
            v0n_sb = const.tile([NB, DIN + 1], bf16)
            nc.sync.dma_start(out=v0n_sb, in_=v_ext[:, 0, :])

            # ---- phase 0: per-batch folded queries (all tiny, fp32) ----
            with tc.tile_pool(name="ps_prep", bufs=2, space="PSUM") as ps_prep:
                # Short HAM warmup: a few matmuls ahead of phase0 get the
                # PE clock-gate opening (1.2->2.4GHz) while the bf16 weight
                # DMA completes; a long warmup would delay phase0 in the PE
                # FIFO instead (measured: 8 ~ 12 beats 4 and 22).
                for wi in range(8):
                    pwarm = ps_prep.tile([128, 256], fp32, tag="pw")
                    nc.tensor.matmul(
                        pwarm[:, 0:128],
                        lhsT=ident_bf,
                        rhs=ident_bf,
                        start=True,
                        stop=True,
                    )

                # WkT[j_p, jc, din] = W_k.T via PE transpose
                wkT_sb = const.tile([128, 2, 256], bf16)
                for jc in range(2):
                    pw = ps_prep.tile([128, 256], fp32, tag="pw")
                    for kc in range(2):
                        nc.tensor.matmul(
                            pw[:, kc * 128 : (kc + 1) * 128],
                            lhsT=wk_sb[:, kc, jc * 128 : (jc + 1) * 128],
                            rhs=ident_bf,
                            start=True,
                            stop=True,
                        )
                    nc.vector.tensor_copy(out=wkT_sb[:, jc, :], in_=pw)

                # bq[din_p, kc] and v0T[din_p, kc, n] via PE row->column
                # transposes (shared PSUM tile: col 0 = b_q, cols 1.. = v0)
                bv_sb = const.tile([128, 2, 1 + NB], fp32)
                pbv = ps_prep.tile([128, 2, 1 + NB], fp32, tag="pbv")
                for kc in range(2):
                    nc.tensor.matmul(
                        pbv[:, kc, 0:1],
                        lhsT=bqn_sb[:, kc * 128 : (kc + 1) * 128],
                        rhs=ident[0:1, 0:1],
                        start=True,
                        stop=True,
                    )
                    nc.tensor.matmul(
                        pbv[:, kc, 1 : 1 + NB],
                        lhsT=v0n_sb[:, kc * 128 : (kc + 1) * 128],
                        rhs=ident_bf[0:NB, 0:NB],
                        start=True,
                        stop=True,
                    )
                nc.vector.tensor_copy(out=bv_sb, in_=pbv)
                v0b_sb = const.tile([128, 2, NB], bf16)
                nc.vector.tensor_copy(out=v0b_sb, in_=pbv[:, :, 1 : 1 + NB])

                # q0[dq_p, dqc, n] = W_q.T @ v0 + b_q  (batched over n)
                q0_sb = const.tile([128, 2, NB], fp32)
                for dqc in range(2):
                    pq = ps_prep.tile([128, NB], fp32, tag="pq")
                    for kc in range(2):
                        nc.tensor.matmul(
                            pq,
                            lhsT=wq_sb[:, kc, dqc * 128 : (dqc + 1) * 128],
                            rhs=v0b_sb[:, kc, :],
                            start=(kc == 0),
                            stop=(kc == 1),
                        )
                    nc.scalar.activation(
                        out=q0_sb[:, dqc, :],
                        in_=pq,
                        func=AF.Identity,
                        bias=bv_sb[:, dqc, 0:1],
                        scale=1.0,
                    )

                # head mask[j_p, jc, h] = SCALE where j = 128*jc + j_p lies in
                # head h's 32-slice, else 0  (j - 32h in [0, 32))
                mask_sb = const.tile([128, 2, H], fp32)
                nc.gpsimd.memset(mask_sb, SCALE)
                nc.gpsimd.affine_select(
                    out=mask_sb,
                    in_=mask_sb,
                    compare_op=mybir.AluOpType.is_ge,
                    fill=0.0,
                    base=0,
                    pattern=[[128, 2], [-32, H]],
                    channel_multiplier=1,
                )
                nc.gpsimd.affine_select(
                    out=mask_sb,
                    in_=mask_sb,
                    compare_op=mybir.AluOpType.is_ge,
                    fill=0.0,
                    base=31,
                    pattern=[[-128, 2], [32, H]],
                    channel_multiplier=-1,
                )

                # q0m[j_p, jc, n*8+h] = mask * q0 (per-partition scalar)
                q0m_sb = const.tile([128, 2, NB * H], bf16)
                for n in range(NB):
                    for jc in range(2):
                        nc.vector.tensor_scalar_mul(
                            q0m_sb[:, jc, n * H : (n + 1) * H],
                            mask_sb[:, jc, :],
                            q0_sb[:, jc, n : n + 1],
                        )

                # fq[din_p, kc, n*8+h] = W_k @ q0m  (lhsT = WkT, batched over n)
                fq_bf = const.tile([128, 2, NB * H], bf16)
                for kc in range(2):
                    pf = ps_prep.tile([128, NB * H], fp32, tag="pf")
                    for jc in range(2):
                        nc.tensor.matmul(
                            pf,
                            lhsT=wkT_sb[:, jc, kc * 128 : (kc + 1) * 128],
                            rhs=q0m_sb[:, jc, :],
                            start=(jc == 0),
                            stop=(jc == 1),
                        )
                    nc.vector.tensor_copy(out=fq_bf[:, kc, :], in_=pf)

            # ---- phase 1: stream v ----
            vbf = ctx.enter_context(tc.tile_pool(name="vbf", bufs=5))
            vt = ctx.enter_context(tc.tile_pool(name="vt", bufs=4))
            et = ctx.enter_context(tc.tile_pool(name="et", bufs=4))
            work = ctx.enter_context(tc.tile_pool(name="work", bufs=2))
            ps_t = ctx.enter_context(tc.tile_pool(name="ps_t", bufs=6, space="PSUM"))
            ps_s = ctx.enter_context(tc.tile_pool(name="ps_s", bufs=1, space="PSUM"))
            ps_o = ctx.enter_context(tc.tile_pool(name="ps_o", bufs=1, space="PSUM"))

            res_sb = const.tile([H, NB, DIN], fp32)

            state = {"oacc": None}
            pending = []

            def value_stage(et_sb, vbf_sb, n, ci):
                # value: out_acc[h, 0:256] += e.T @ v ; col 256 accumulates se
                if ci == 0:
                    oacc = ps_o.tile([H, DIN + 1], fp32, tag="oacc")
                    state["oacc"] = oacc
                oacc = state["oacc"]
                for j in range(NJ):
                    nc.tensor.matmul(
                        oacc,
                        lhsT=et_sb[:, j, :],
                        rhs=vbf_sb[:, j, :],
                        start=(ci == 0 and j == 0),
                        stop=(ci == NCH - 1 and j == NJ - 1),
                    )
                if ci == NCH - 1:
                    rec = work.tile([H, 1], fp32, tag="rec")
                    nc.vector.reciprocal(out=rec, in_=oacc[:, DIN : DIN + 1])
                    nc.vector.tensor_scalar_mul(res_sb[:, n, :], oacc[:, 0:DIN], rec)
                    # stream each batch's result out as soon as it is ready
                    nc.sync.dma_start(
                        out=out_ext[n, :, :].rearrange("h (o d) -> h o d", o=1),
                        in_=res_sb[:, n : n + 1, :],
                    )

            vpair = None
            for gi in range(GCH):
                n, ci = divmod(gi, NCH)
                half = ci % 2
                if half == 0:
                    # paired p-major DMA over 2 chunks: [t_p, jj, din+1],
                    # t = ci*TC + 8*t_p + jj — one contiguous ~4KB HBM
                    # segment per partition (large DMA packets).  Column 256
                    # carries the host-prepended ones.
                    t0 = ci * TC
                    vpair = vbf.tile([128, 2 * NJ, DIN + 1], bf16, tag="vbf")
                    nc.sync.dma_start(
                        out=vpair,
                        in_=v_ext[n, t0 : t0 + 2 * TC, :].rearrange(
                            "(p jj) d -> p jj d", p=128
                        ),
                    )
                vbf_sb = vpair[:, half * NJ : (half + 1) * NJ, :]

                # vT[din_p, kc, (j p)] via PE (regular bf16 matmul w/ identity)
                # one PSUM tile (= one bank) per kc half -> deeper pipelining
                vt_sb = vt.tile([128, 2, TC], bf16, tag="vt")
                for kc in range(2):
                    pvt = ps_t.tile([128, TC], fp32, tag="pvt")
                    for j in range(NJ):
                        nc.tensor.matmul(
                            pvt[:, j * 128 : (j + 1) * 128],
                            lhsT=vbf_sb[:, j, kc * 128 : (kc + 1) * 128],
                            rhs=ident_bf,
                            start=True,
                            stop=True,
                        )
                    if kc == 0:
                        nc.vector.tensor_copy(out=vt_sb[:, kc, :], in_=pvt)
                    else:
                        nc.scalar.copy(out=vt_sb[:, kc, :], in_=pvt)

                # sT[t_p, j, h] = vT_block.T @ fq  (vT stationary, fq moving)
                ps = ps_s.tile([128, NJ, H], fp32, tag="ps")
                for j in range(NJ):
                    for kc in range(2):
                        nc.tensor.matmul(
                            ps[:, j, :],
                            lhsT=vt_sb[:, kc, j * 128 : (j + 1) * 128],
                            rhs=fq_bf[:, kc, n * H : (n + 1) * H],
                            start=(kc == 0),
                            stop=(kc == 1),
                        )

                # eT[t_p, j, h] = exp(sT)   (score offset c cancels in softmax)
                et_sb = et.tile([128, NJ, H], bf16, tag="et")
                nc.scalar.activation(out=et_sb, in_=ps, func=AF.Exp, scale=1.0)

                # value stage is emitted one chunk late: V(i) waits on exp(i),
                # and in PE FIFO order it would block T(i+1) while waiting
                pending.append((et_sb, vbf_sb, n, ci))
                if len(pending) > 1:
                    value_stage(*pending.pop(0))
            while pending:
                value_stage(*pending.pop(0))

    nc.compile()
    return nc


def _get_nc():
    if "nc" not in _CACHE:
        _CACHE["nc"] = _build()
    return _CACHE["nc"]


def _run(inputs, trace=False):
    import ml_dtypes

    from concourse.bass_utils import run_bass_kernel_spmd

    v = np.asarray(inputs["v"])
    w = np.ascontiguousarray(
        np.asarray(inputs["W_qk"], dtype=np.float32).astype(ml_dtypes.bfloat16)
    )
    b = np.ascontiguousarray(np.asarray(inputs["b_qk"], dtype=np.float32))
    # bf16 upload with a ones column at index 256: halves HBM traffic and
    # feeds the softmax-denominator column (device computes in bf16 anyway)
    vb = np.empty((N_FULL, T, DIN + 1), dtype=ml_dtypes.bfloat16)
    vb[:, :, 0:DIN] = v.astype(ml_dtypes.bfloat16)
    vb[:, :, DIN] = 1.0
    nc = _get_nc()
    in_maps = [
        {"v": vb[c * NB : (c + 1) * NB], "W_qk": w, "b_qk": b} for c in range(NCORES)
    ]
    res = run_bass_kernel_spmd(nc, in_maps, list(range(NCORES)), trace=trace)
    full = np.concatenate(
        [res.results[c]["out"] for c in range(NCORES)], axis=0
    )  # [N, H, DIN]
    # out[n, 32h + i] = full[n, h, 32h + i]
    fh = full.reshape(N_FULL, H, H, 32)  # [n, h, h', i]
    out = np.ascontiguousarray(
        fh[:, np.arange(H), np.arange(H), :].reshape(N_FULL, DIN)
    ).astype(np.float32)
    return out, res


def kernel(**inputs) -> np.ndarray:
    return _run(inputs, trace=False)[0]

